# revision 7
# baseline (speedup 1.0000x reference)
"""Distributed GCN(4-layer) + LSTM readout kernel for 8 TRN2 NeuronCores.

Self-contained: hardcodes the problem shapes (N=50000, E=800000, D=H=128,
G=500 graphs x L=100 nodes, C=10) and the 8-way sharding.

Strategy
--------
- Nodes are sharded contiguously across 8 cores at graph boundaries
  (sizes 6300 x4 + 6200 x4), so the per-graph LSTM readout is purely local.
- Per GCN layer, each core computes u = t @ W for its shard in [node, feat]
  layout (t carries the deg^-1/2 row prescale: norm separability
  a[src]*a[dst] -> prescale rows, post-scale dst blocks), writes the slab to
  DRAM and AllGathers the 8 slabs into a replicated bf16 table.
- Edge aggregation: edges partitioned by dst shard, grouped by 128-dst
  blocks. The scatter-add becomes PSUM matmul accumulation:
  psum[f,d] += sum_e GX[e,f] * S[e,d], with GX = dma_gather of table rows
  (by src) and S a 0/1 staircase built on the vector engine via
  is_equal(seg, iota). The self-loop enters as one identity matmul of the
  local slab block. Post per block: h = relu(a*psum + b), t_next = a*h.
- dma_gather indices are int16, so the table is addressed in two halves
  (cores 0-3 / 4-7) and per-block edge lists are split accordingly.
- LSTM: x-projections for all timesteps are 4 big matmuls up front; the
  recurrence runs 4 [128x128]@[128,63] matmuls + gate math per step.
"""
import dataclasses
import os
import numpy as np
import ml_dtypes

import concourse.bass as bass
import concourse.mybir as mybir
import concourse.tile as tile
from concourse import bacc
from concourse.bass_utils import run_bass_kernel_spmd

F32 = mybir.dt.float32
BF16 = mybir.dt.bfloat16
I16 = mybir.dt.int16
P = 128

TRACE = False          # set True (e.g. from test.py) to profile
LAST_RESULTS = None    # BassKernelResults of the last run (for profiling)


@dataclasses.dataclass
class Config:
    N: int = 50000
    E: int = 800000
    D: int = 128
    H: int = 128
    L: int = 100
    C: int = 10
    NCORES: int = 8
    GROUP_BLOCKS: int = 4  # dst blocks per gather super-group

    def __post_init__(self):
        assert self.D == 128 and self.H == 128
        base = (self.N // self.NCORES) // self.L * self.L
        hi = base + self.L
        n_hi = (self.N - base * self.NCORES) // self.L
        self.sizes = [hi] * n_hi + [base] * (self.NCORES - n_hi)
        assert sum(self.sizes) == self.N
        self.offs = np.concatenate([[0], np.cumsum(self.sizes)]).astype(np.int64)
        self.S_PAD = hi
        self.NBLK = -(-self.S_PAD // P)
        self.SLAB = self.NBLK * P
        self.THALF = (self.NCORES // 2) * self.SLAB
        assert self.THALF <= 32768, "int16 gather index overflow"
        self.NG = self.S_PAD // self.L
        self.G = self.N // self.L


PAD_SEG = 255.0


def preprocess(cfg, x, edge_index, Ws, bs, W_ih, W_hh, b_ih, b_hh,
               lin_W, lin_b):
    N = cfg.N
    src = np.asarray(edge_index[0], np.int64)
    dst = np.asarray(edge_index[1], np.int64)
    deg = (np.bincount(dst, minlength=N) + 1.0).astype(np.float32)
    a = (1.0 / np.sqrt(deg)).astype(np.float32)

    shard_of = np.searchsorted(cfg.offs[1:], np.arange(N), side="right")
    trow = shard_of * cfg.SLAB + (np.arange(N) - cfg.offs[shard_of])

    e_core = shard_of[dst]
    e_half = (trow[src] >= cfg.THALF).astype(np.int64)
    e_tix = (trow[src] - e_half * cfg.THALF).astype(np.int64)
    e_blk = ((dst - cfg.offs[e_core]) // P).astype(np.int64)
    e_seg = ((dst - cfg.offs[e_core]) % P).astype(np.int64)

    order = np.lexsort((dst, e_blk, e_half, e_core))
    src_s = src[order]
    core_s, half_s, tix_s, blk_s, seg_s = (
        arr[order] for arr in (e_core, e_half, e_tix, e_blk, e_seg))

    counts = np.zeros((cfg.NCORES, 2, cfg.NBLK), np.int64)
    np.add.at(counts, (core_s, half_s, blk_s), 1)
    chunks = -(-counts.max(axis=0) // P)
    cA, cB = chunks[0], chunks[1]

    groups = []
    for g0 in range(0, cfg.NBLK, cfg.GROUP_BLOCKS):
        groups.append(list(range(g0, min(g0 + cfg.GROUP_BLOCKS, cfg.NBLK))))

    TA = int(cA.sum()) * P
    TB = int(cB.sum()) * P
    run_off = np.zeros((cfg.NCORES, 2, cfg.NBLK), np.int64)
    run_off.reshape(-1)[1:] = np.cumsum(counts.reshape(-1))[:-1]

    in_maps = []
    for c in range(cfg.NCORES):
        idx_flat = {0: np.zeros(TA, np.int64), 1: np.zeros(TB, np.int64)}
        seg_flat = {0: np.full(TA, PAD_SEG, np.float32),
                    1: np.full(TB, PAD_SEG, np.float32)}
        for h_i, tot_c in enumerate((cA, cB)):
            pos = 0
            for b in range(cfg.NBLK):
                n = int(counts[c, h_i, b])
                o = int(run_off[c, h_i, b])
                idx_flat[h_i][pos:pos + n] = tix_s[o:o + n]
                seg_flat[h_i][pos:pos + n] = seg_s[o:o + n]
                pos += int(tot_c[b]) * P
        idxA = np.zeros((P, max(TA // 16, 1)), np.int16)
        idxB = np.zeros((P, max(TB // 16, 1)), np.int16)
        if TA:
            idxA[:] = np.tile(idx_flat[0].reshape(-1, 16).T.astype(np.int16),
                              (8, 1))
        if TB:
            idxB[:] = np.tile(idx_flat[1].reshape(-1, 16).T.astype(np.int16),
                              (8, 1))
        segA = np.zeros((P, max(TA // P, 1)), ml_dtypes.bfloat16)
        segB = np.zeros((P, max(TB // P, 1)), ml_dtypes.bfloat16)
        if TA:
            segA[:] = seg_flat[0].reshape(-1, P).T.astype(ml_dtypes.bfloat16)
        if TB:
            segB[:] = seg_flat[1].reshape(-1, P).T.astype(ml_dtypes.bfloat16)

        o, s = int(cfg.offs[c]), int(cfg.sizes[c])
        xT = np.zeros((P, cfg.SLAB), np.float32)
        xT[:, :s] = np.asarray(x[o:o + s], np.float32).T
        a_b = np.ones((P, cfg.SLAB), np.float32)
        a_b[:, :s] = np.tile(a[o:o + s], (P, 1))

        m = {
            "xT": xT, "a_b": a_b,
            "idxA": idxA, "idxB": idxB, "segA": segA, "segB": segB,
            "iota": np.tile(np.arange(P, dtype=np.float32), (P, 1)).astype(
                ml_dtypes.bfloat16),
            "ident": np.eye(P, dtype=ml_dtypes.bfloat16),
            "WihT": np.ascontiguousarray(
                np.asarray(W_ih, np.float32).T).astype(ml_dtypes.bfloat16),
            "WhhT": np.ascontiguousarray(np.asarray(W_hh, np.float32).T),
            "bg": np.ascontiguousarray(
                (np.asarray(b_ih, np.float32)
                 + np.asarray(b_hh, np.float32)).reshape(4, P).T),
            "linW": np.asarray(lin_W, np.float32),
            "linb": np.tile(np.asarray(lin_b, np.float32), (P, 1)),
        }
        for li in range(4):
            m[f"W{li}"] = np.asarray(Ws[li], np.float32).astype(
                ml_dtypes.bfloat16)
            m[f"bias{li}"] = np.asarray(bs[li], np.float32).reshape(P, 1)
        in_maps.append(m)

    sched = dict(cA=cA.astype(np.int64), cB=cB.astype(np.int64),
                 groups=groups, TA=TA, TB=TB)
    return in_maps, sched


def build_program(cfg, sched, trn_type="TRN2", debug=False):
    dbg_layers = int(os.environ.get("GNN_LAYERS", "4"))
    dbg_skip_edges = os.environ.get("GNN_SKIP_EDGES", "0") == "1"
    dbg_skip_gather = os.environ.get("GNN_SKIP_GATHER", "0") == "1"
    dbg_skip_lstm = os.environ.get("GNN_SKIP_LSTM", "0") == "1"
    dbg_skip_coll = os.environ.get("GNN_SKIP_COLL", "0") == "1"
    nc = bacc.Bacc(trn_type, target_bir_lowering=False, debug=debug,
                   num_devices=cfg.NCORES)
    cA, cB, groups = sched["cA"], sched["cB"], sched["groups"]
    TA, TB = sched["TA"], sched["TB"]
    SLAB, NBLK, THALF, NG = cfg.SLAB, cfg.NBLK, cfg.THALF, cfg.NG

    def din(name, shape, dt):
        return nc.dram_tensor(name, shape, dt, kind="ExternalInput")

    xT_d = din("xT", [P, SLAB], F32)
    a_d = din("a_b", [P, SLAB], F32)
    idxA_d = din("idxA", [P, max(TA // 16, 1)], I16)
    idxB_d = din("idxB", [P, max(TB // 16, 1)], I16)
    segA_d = din("segA", [P, max(TA // P, 1)], BF16)
    segB_d = din("segB", [P, max(TB // P, 1)], BF16)
    iota_d = din("iota", [P, P], BF16)
    ident_d = din("ident", [P, P], BF16)
    W_d = [din(f"W{li}", [P, P], BF16) for li in range(4)]
    bias_d = [din(f"bias{li}", [P, 1], F32) for li in range(4)]
    WihT_d = din("WihT", [P, 4 * P], BF16)
    WhhT_d = din("WhhT", [P, 4 * P], F32)
    bg_d = din("bg", [P, 4], F32)
    linW_d = din("linW", [P, cfg.C], F32)
    linb_d = din("linb", [P, cfg.C], F32)
    out_d = nc.dram_tensor("out", [NG, cfg.C], F32, kind="ExternalOutput")

    rg = [list(range(cfg.NCORES))]

    with tile.TileContext(nc) as tc:
        with tc.tile_pool(name="dram", bufs=1, space="DRAM") as dpool, \
             tc.tile_pool(name="const", bufs=1) as cpool, \
             tc.tile_pool(name="state", bufs=1) as spool, \
             tc.tile_pool(name="work", bufs=2) as wpool, \
             tc.tile_pool(name="psum", bufs=4, space="PSUM") as ppool:


            def cload(dram, shape, dt, tag):
                t = cpool.tile(shape, dt, tag=tag)
                nc.sync.dma_start(t[:], dram[:])
                return t
            iota_t = cload(iota_d, [P, P], BF16, "c_iota")
            ident_t = cload(ident_d, [P, P], BF16, "c_ident")
            W_t = [cload(W_d[i], [P, P], BF16, f"c_W{i}") for i in range(4)]
            bias_t = [cload(bias_d[i], [P, 1], F32, f"c_b{i}")
                      for i in range(4)]
            WihT_t = cload(WihT_d, [P, 4 * P], BF16, "c_wih")
            WhhT_t = cload(WhhT_d, [P, 4 * P], F32, "c_whh")
            bg_t = cload(bg_d, [P, 4], F32, "c_bg")
            linW_t = cload(linW_d, [P, cfg.C], F32, "c_linw")
            linb_t = cload(linb_d, [P, cfg.C], F32, "c_linb")

            a_t = spool.tile([P, SLAB], F32, tag="a")
            nc.sync.dma_start(a_t[:], a_d[:])

            t_even = spool.tile([P, SLAB], BF16, tag="t_even")
            t_big = spool.tile([P, 4 * SLAB], BF16, tag="t_big")
            slab_t = spool.tile([P, SLAB], BF16, tag="slab")

            for j in range(0, SLAB, 512):
                w = min(512, SLAB - j)
                xc = wpool.tile([P, 512], F32, tag="xchunk")
                nc.sync.dma_start(xc[:, :w], xT_d[:, j:j + w])
                nc.vector.tensor_tensor(
                    out=t_even[:, j:j + w], in0=xc[:, :w], in1=a_t[:, j:j + w],
                    op=mybir.AluOpType.mult)

            for li in range(4):
                cur = t_even if li % 2 == 0 else t_big
                nxt = t_big if li % 2 == 0 else t_even
                slab_dram = dpool.tile([SLAB, P], BF16, tag="slab_dram",
                                       bufs=2)
                table_dram = dpool.tile([cfg.NCORES * SLAB, P], BF16,
                                        addr_space="Shared", tag="table",
                                        bufs=2)

                # ---- u = t @ W -> slab [node, feat] ----
                for k in range(NBLK):
                    pu = ppool.tile([P, P], F32, tag="mm", space="PSUM")
                    nc.tensor.matmul(pu[:], lhsT=cur[:, k * P:(k + 1) * P],
                                     rhs=W_t[li][:], start=True, stop=True)
                    nc.vector.tensor_copy(out=slab_t[:, k * P:(k + 1) * P],
                                          in_=pu[:])
                nc.sync.dma_start(
                    slab_dram[:].rearrange("(b p) f -> p b f", p=P),
                    slab_t[:].rearrange("p (b f) -> p b f", f=P))
                if dbg_skip_coll:
                    nc.sync.dma_start(
                        table_dram[li % 2 * SLAB:(li % 2 + 1) * SLAB, :],
                        slab_dram[:])
                else:
                    nc.gpsimd.collective_compute(
                        "AllGather", mybir.AluOpType.bypass,
                        replica_groups=rg,
                        ins=[slab_dram[:]],
                        outs=[table_dram[:]],
                    )

                # ---- edge aggregation ----
                if li >= dbg_layers:
                    continue
                ao = 0
                bo = 0
                if dbg_skip_edges:
                    continue
                for blks in groups:
                    nca = int(cA[blks].sum())
                    ncb = int(cB[blks].sum())
                    gx = {}
                    for half, ncnt, idxd, off16 in (
                            (0, nca, idxA_d, ao), (1, ncb, idxB_d, bo)):
                        if ncnt == 0:
                            continue
                        it = wpool.tile([P, ncnt * 8], I16, tag=f"idx{half}")
                        nc.sync.dma_start(
                            it[:], idxd[:, off16 * 8:(off16 + ncnt) * 8])
                        g = wpool.tile([P, ncnt, P], BF16, tag=f"gx{half}")
                        if dbg_skip_gather:
                            nc.vector.memset(g[:], 0.0)
                            gx[half] = g
                            continue
                        nc.gpsimd.dma_gather(
                            out_ap=g[:],
                            in_ap=table_dram[half * THALF:(half + 1) * THALF, :],
                            idxs_ap=it[:],
                            num_idxs=ncnt * P,
                            num_idxs_reg=ncnt * P,
                            elem_size=P,
                            single_packet=False,
                        )
                        gx[half] = g
                    sa = {}
                    for half, ncnt, segd, offc in (
                            (0, nca, segA_d, ao), (1, ncb, segB_d, bo)):
                        if ncnt == 0:
                            continue
                        st_ = wpool.tile([P, ncnt], BF16, tag=f"seg{half}")
                        nc.sync.dma_start(st_[:], segd[:, offc:offc + ncnt])
                        sa[half] = st_

                    ca_in_grp = 0
                    cb_in_grp = 0
                    for b in blks:
                        pb = ppool.tile([P, P], F32, tag="mm", space="PSUM")
                        na, nb_ = int(cA[b]), int(cB[b])
                        nc.tensor.matmul(pb[:],
                                         lhsT=slab_t[:, b * P:(b + 1) * P],
                                         rhs=ident_t[:], start=True,
                                         stop=(na + nb_ == 0))
                        done = 0
                        for half, cnt, base in ((0, na, ca_in_grp),
                                                (1, nb_, cb_in_grp)):
                            for ci in range(cnt):
                                col = base + ci
                                st = wpool.tile([P, P], BF16, tag="st",
                                                bufs=4)
                                nc.vector.tensor_tensor(
                                    out=st[:],
                                    in0=sa[half][:, col:col + 1]
                                        .to_broadcast((P, P)),
                                    in1=iota_t[:],
                                    op=mybir.AluOpType.is_equal)
                                done += 1
                                nc.tensor.matmul(
                                    pb[:], lhsT=gx[half][:, col, :], rhs=st[:],
                                    start=False, stop=(done == na + nb_))
                        ca_in_grp += na
                        cb_in_grp += nb_

                        tmp = wpool.tile([P, P], F32, tag="tmp")
                        nc.vector.tensor_tensor(
                            out=tmp[:], in0=pb[:],
                            in1=a_t[:, b * P:(b + 1) * P],
                            op=mybir.AluOpType.mult)
                        if li < 3:
                            z = wpool.tile([P, P], F32, tag="z")
                            nc.scalar.activation(
                                out=z[:], in_=tmp[:],
                                func=mybir.ActivationFunctionType.Relu,
                                bias=bias_t[li][:])
                            nc.vector.tensor_tensor(
                                out=nxt[:, b * P:(b + 1) * P], in0=z[:],
                                in1=a_t[:, b * P:(b + 1) * P],
                                op=mybir.AluOpType.mult)
                        else:
                            nc.scalar.activation(
                                out=nxt[:, b * P:(b + 1) * P], in_=tmp[:],
                                func=mybir.ActivationFunctionType.Relu,
                                bias=bias_t[li][:])
                    ao += nca
                    bo += ncb

            z4 = t_even
            gatesx = t_big  # reuse [P, 4*SLAB] bf16

            for q in range(4) if not dbg_skip_lstm else []:
                for j in range(0, SLAB, 512):
                    w = min(512, SLAB - j)
                    pgx = ppool.tile([P, 512], F32, tag="mm", space="PSUM")
                    nc.tensor.matmul(
                        pgx[:, :w], lhsT=WihT_t[:, q * P:(q + 1) * P],
                        rhs=z4[:, j:j + w], start=True, stop=True)
                    nc.vector.tensor_copy(
                        out=gatesx[:, q * SLAB + j:q * SLAB + j + w],
                        in_=pgx[:, :w])

            if dbg_skip_lstm:
                dummy = wpool.tile([P, cfg.C], F32, tag="outs")
                nc.vector.memset(dummy[:], 0.0)
                nc.sync.dma_start(out_d[:], dummy[:NG, :])
            c_t = spool.tile([P, NG], F32, tag="c")
            h_t = spool.tile([P, NG], F32, tag="h")
            nc.vector.memset(c_t[:], 0.0)
            nc.vector.memset(h_t[:], 0.0)

            for t in range(cfg.L) if not dbg_skip_lstm else []:
                pg = ppool.tile([P, 4 * NG], F32, tag="lstm", space="PSUM",
                                bufs=2)
                for q in range(4):
                    nc.tensor.matmul(
                        pg[:, q * NG:(q + 1) * NG],
                        lhsT=WhhT_t[:, q * P:(q + 1) * P],
                        rhs=h_t[:], start=True, stop=True)
                gsum = wpool.tile([P, 4 * NG], F32, tag="gsum")
                for q in range(4):
                    nc.vector.tensor_tensor(
                        out=gsum[:, q * NG:(q + 1) * NG],
                        in0=pg[:, q * NG:(q + 1) * NG],
                        in1=gatesx[:, q * SLAB + t:q * SLAB + cfg.S_PAD:cfg.L],
                        op=mybir.AluOpType.add)
                acts = []
                for q, fn in enumerate((
                        mybir.ActivationFunctionType.Sigmoid,
                        mybir.ActivationFunctionType.Sigmoid,
                        mybir.ActivationFunctionType.Tanh,
                        mybir.ActivationFunctionType.Sigmoid)):
                    av = wpool.tile([P, NG], F32, tag=f"act{q}")
                    nc.scalar.activation(
                        out=av[:], in_=gsum[:, q * NG:(q + 1) * NG],
                        func=fn, bias=bg_t[:, q:q + 1])
                    acts.append(av)
                i_t, f_t, g_t, o_t = acts
                fc = wpool.tile([P, NG], F32, tag="fc")
                nc.vector.tensor_tensor(out=fc[:], in0=f_t[:], in1=c_t[:],
                                        op=mybir.AluOpType.mult)
                ig = wpool.tile([P, NG], F32, tag="ig")
                nc.vector.tensor_tensor(out=ig[:], in0=i_t[:], in1=g_t[:],
                                        op=mybir.AluOpType.mult)
                nc.vector.tensor_tensor(out=c_t[:], in0=fc[:], in1=ig[:],
                                        op=mybir.AluOpType.add)
                tc_ = wpool.tile([P, NG], F32, tag="tc")
                nc.scalar.activation(out=tc_[:], in_=c_t[:],
                                     func=mybir.ActivationFunctionType.Tanh)
                nc.vector.tensor_tensor(out=h_t[:], in0=o_t[:], in1=tc_[:],
                                        op=mybir.AluOpType.mult)

            if not dbg_skip_lstm:
                po = ppool.tile([P, cfg.C], F32, tag="lstm", space="PSUM",
                                bufs=2)
                nc.tensor.matmul(po[:NG, :], lhsT=h_t[:, :NG], rhs=linW_t[:],
                                 start=True, stop=True)
                os_ = wpool.tile([P, cfg.C], F32, tag="outs")
                nc.vector.tensor_tensor(out=os_[:NG, :], in0=po[:NG, :],
                                        in1=linb_t[:NG, :],
                                        op=mybir.AluOpType.add)
                nc.sync.dma_start(out_d[:], os_[:NG, :])

    nc.compile()
    return nc


def assemble(cfg, results):
    out = np.zeros((cfg.G, cfg.C), np.float32)
    for c in range(cfg.NCORES):
        g0 = int(cfg.offs[c]) // cfg.L
        ng = cfg.sizes[c] // cfg.L
        out[g0:g0 + ng] = results[c]["out"][:ng]
    return out


_BUILD_CACHE = {}


def kernel(x, edge_index, batch, W1, b1, W2, b2, W3, b3, W4, b4,
           W_ih, W_hh, b_ih, b_hh, lin_W, lin_b):
    global LAST_RESULTS
    cfg = Config()
    x = np.asarray(x, np.float32)
    edge_index = np.asarray(edge_index, np.int64)
    Ws = [np.asarray(w, np.float32) for w in (W1, W2, W3, W4)]
    bs = [np.asarray(b, np.float32) for b in (b1, b2, b3, b4)]

    in_maps, sched = preprocess(
        cfg, x, edge_index, Ws, bs,
        np.asarray(W_ih, np.float32), np.asarray(W_hh, np.float32),
        np.asarray(b_ih, np.float32), np.asarray(b_hh, np.float32),
        np.asarray(lin_W, np.float32), np.asarray(lin_b, np.float32))

    key = (sched["TA"], sched["TB"], tuple(sched["cA"]), tuple(sched["cB"]))
    if key not in _BUILD_CACHE:
        _BUILD_CACHE[key] = build_program(cfg, sched)
    nc = _BUILD_CACHE[key]

    res = run_bass_kernel_spmd(nc, in_maps, core_ids=list(range(cfg.NCORES)),
                               trace=TRACE)
    LAST_RESULTS = res
    return assemble(cfg, res.results)


# revision 8
# speedup vs baseline: 1.4670x; 1.4670x over previous
"""Distributed GCN(4-layer) + LSTM readout kernel for 8 TRN2 NeuronCores.

Self-contained: hardcodes the problem shapes (N=50000, E=800000, D=H=128,
G=500 graphs x L=100 nodes, C=10) and the 8-way sharding.

Strategy
--------
- Nodes are sharded contiguously across 8 cores at graph boundaries
  (sizes 6300 x4 + 6200 x4), so the per-graph LSTM readout is purely local.
- Per GCN layer, each core computes u = t @ W for its shard in [node, feat]
  layout (t carries the deg^-1/2 row prescale: norm separability
  a[src]*a[dst] -> prescale rows, post-scale dst blocks), writes the slab to
  DRAM and AllGathers the 8 slabs into a replicated bf16 table.
- Edge aggregation: edges partitioned by dst shard, grouped by 128-dst
  blocks. The scatter-add becomes PSUM matmul accumulation:
  psum[f,d] += sum_e GX[e,f] * S[e,d], with GX = dma_gather of table rows
  (by src) and S a 0/1 staircase built on the vector engine via
  is_equal(seg, iota). The self-loop enters as one identity matmul of the
  local slab block. Post per block: h = relu(a*psum + b), t_next = a*h.
- dma_gather indices are int16, so the table is addressed in two halves
  (cores 0-3 / 4-7) and per-block edge lists are split accordingly.
- LSTM: x-projections for all timesteps are 4 big matmuls up front; the
  recurrence runs 4 [128x128]@[128,63] matmuls + gate math per step.
"""
import dataclasses
import os
import numpy as np
import ml_dtypes

import concourse.bass as bass
import concourse.mybir as mybir
import concourse.tile as tile
from concourse import bacc
from concourse.bass_utils import run_bass_kernel_spmd

F32 = mybir.dt.float32
BF16 = mybir.dt.bfloat16
I16 = mybir.dt.int16
P = 128

TRACE = False          # set True (e.g. from test.py) to profile
LAST_RESULTS = None    # BassKernelResults of the last run (for profiling)


@dataclasses.dataclass
class Config:
    N: int = 50000
    E: int = 800000
    D: int = 128
    H: int = 128
    L: int = 100
    C: int = 10
    NCORES: int = 8
    GROUP_BLOCKS: int = 4  # dst blocks per gather super-group

    def __post_init__(self):
        assert self.D == 128 and self.H == 128
        base = (self.N // self.NCORES) // self.L * self.L
        hi = base + self.L
        n_hi = (self.N - base * self.NCORES) // self.L
        self.sizes = [hi] * n_hi + [base] * (self.NCORES - n_hi)
        assert sum(self.sizes) == self.N
        self.offs = np.concatenate([[0], np.cumsum(self.sizes)]).astype(np.int64)
        self.S_PAD = hi
        self.NBLK = -(-self.S_PAD // P)
        self.SLAB = self.NBLK * P
        self.THALF = (self.NCORES // 2) * self.SLAB
        assert self.THALF <= 32768, "int16 gather index overflow"
        self.NG = self.S_PAD // self.L
        self.G = self.N // self.L


PAD_SEG = 255.0


def preprocess(cfg, x, edge_index, Ws, bs, W_ih, W_hh, b_ih, b_hh,
               lin_W, lin_b):
    N = cfg.N
    src = np.asarray(edge_index[0], np.int64)
    dst = np.asarray(edge_index[1], np.int64)
    deg = (np.bincount(dst, minlength=N) + 1.0).astype(np.float32)
    a = (1.0 / np.sqrt(deg)).astype(np.float32)

    shard_of = np.searchsorted(cfg.offs[1:], np.arange(N), side="right")
    trow = shard_of * cfg.SLAB + (np.arange(N) - cfg.offs[shard_of])

    e_core = shard_of[dst]
    e_half = (trow[src] >= cfg.THALF).astype(np.int64)
    e_tix = (trow[src] - e_half * cfg.THALF).astype(np.int64)
    e_blk = ((dst - cfg.offs[e_core]) // P).astype(np.int64)
    e_seg = ((dst - cfg.offs[e_core]) % P).astype(np.int64)

    order = np.lexsort((dst, e_blk, e_half, e_core))
    src_s = src[order]
    core_s, half_s, tix_s, blk_s, seg_s = (
        arr[order] for arr in (e_core, e_half, e_tix, e_blk, e_seg))

    counts = np.zeros((cfg.NCORES, 2, cfg.NBLK), np.int64)
    np.add.at(counts, (core_s, half_s, blk_s), 1)
    chunks = -(-counts.max(axis=0) // P)
    cA, cB = chunks[0], chunks[1]

    groups = []
    for g0 in range(0, cfg.NBLK, cfg.GROUP_BLOCKS):
        groups.append(list(range(g0, min(g0 + cfg.GROUP_BLOCKS, cfg.NBLK))))

    TA = int(cA.sum()) * P
    TB = int(cB.sum()) * P
    run_off = np.zeros((cfg.NCORES, 2, cfg.NBLK), np.int64)
    run_off.reshape(-1)[1:] = np.cumsum(counts.reshape(-1))[:-1]

    in_maps = []
    for c in range(cfg.NCORES):
        idx_flat = {0: np.zeros(TA, np.int64), 1: np.zeros(TB, np.int64)}
        seg_flat = {0: np.full(TA, PAD_SEG, np.float32),
                    1: np.full(TB, PAD_SEG, np.float32)}
        for h_i, tot_c in enumerate((cA, cB)):
            pos = 0
            for b in range(cfg.NBLK):
                n = int(counts[c, h_i, b])
                o = int(run_off[c, h_i, b])
                idx_flat[h_i][pos:pos + n] = tix_s[o:o + n]
                seg_flat[h_i][pos:pos + n] = seg_s[o:o + n]
                pos += int(tot_c[b]) * P
        idxA = np.zeros((P, max(TA // 16, 1)), np.int16)
        idxB = np.zeros((P, max(TB // 16, 1)), np.int16)
        if TA:
            idxA[:] = np.tile(idx_flat[0].reshape(-1, 16).T.astype(np.int16),
                              (8, 1))
        if TB:
            idxB[:] = np.tile(idx_flat[1].reshape(-1, 16).T.astype(np.int16),
                              (8, 1))
        segA = np.zeros((P, max(TA // P, 1)), ml_dtypes.bfloat16)
        segB = np.zeros((P, max(TB // P, 1)), ml_dtypes.bfloat16)
        if TA:
            segA[:] = seg_flat[0].reshape(-1, P).T.astype(ml_dtypes.bfloat16)
        if TB:
            segB[:] = seg_flat[1].reshape(-1, P).T.astype(ml_dtypes.bfloat16)

        o, s = int(cfg.offs[c]), int(cfg.sizes[c])
        xT = np.zeros((P, cfg.SLAB), np.float32)
        xT[:, :s] = np.asarray(x[o:o + s], np.float32).T
        a_b = np.ones((P, cfg.SLAB), np.float32)
        a_b[:, :s] = np.tile(a[o:o + s], (P, 1))

        m = {
            "xT": xT, "a_b": a_b,
            "idxA": idxA, "idxB": idxB, "segA": segA, "segB": segB,
            "iota": np.tile(np.arange(P, dtype=np.float32), (P, 1)).astype(
                ml_dtypes.bfloat16),
            "ident": np.eye(P, dtype=ml_dtypes.bfloat16),
            "WihT": np.ascontiguousarray(
                np.asarray(W_ih, np.float32).T).astype(ml_dtypes.bfloat16),
            "WhhT": np.ascontiguousarray(np.asarray(W_hh, np.float32).T),
            "bg": np.ascontiguousarray(
                (np.asarray(b_ih, np.float32)
                 + np.asarray(b_hh, np.float32)).reshape(4, P).T),
            "linW": np.asarray(lin_W, np.float32),
            "linb": np.tile(np.asarray(lin_b, np.float32), (P, 1)),
        }
        for li in range(4):
            m[f"W{li}"] = np.asarray(Ws[li], np.float32).astype(
                ml_dtypes.bfloat16)
            m[f"bias{li}"] = np.asarray(bs[li], np.float32).reshape(P, 1)
        in_maps.append(m)

    sched = dict(cA=cA.astype(np.int64), cB=cB.astype(np.int64),
                 groups=groups, TA=TA, TB=TB)
    return in_maps, sched


def build_program(cfg, sched, trn_type="TRN2", debug=False):
    dbg_layers = int(os.environ.get("GNN_LAYERS", "4"))
    dbg_skip_edges = os.environ.get("GNN_SKIP_EDGES", "0") == "1"
    dbg_skip_gather = os.environ.get("GNN_SKIP_GATHER", "0") == "1"
    dbg_skip_lstm = os.environ.get("GNN_SKIP_LSTM", "0") == "1"
    dbg_skip_coll = os.environ.get("GNN_SKIP_COLL", "0") == "1"
    nc = bacc.Bacc(trn_type, target_bir_lowering=False, debug=debug,
                   num_devices=cfg.NCORES, num_swdge_queues=4)
    cA, cB, groups = sched["cA"], sched["cB"], sched["groups"]
    TA, TB = sched["TA"], sched["TB"]
    SLAB, NBLK, THALF, NG = cfg.SLAB, cfg.NBLK, cfg.THALF, cfg.NG

    def din(name, shape, dt):
        return nc.dram_tensor(name, shape, dt, kind="ExternalInput")

    xT_d = din("xT", [P, SLAB], F32)
    a_d = din("a_b", [P, SLAB], F32)
    idxA_d = din("idxA", [P, max(TA // 16, 1)], I16)
    idxB_d = din("idxB", [P, max(TB // 16, 1)], I16)
    segA_d = din("segA", [P, max(TA // P, 1)], BF16)
    segB_d = din("segB", [P, max(TB // P, 1)], BF16)
    iota_d = din("iota", [P, P], BF16)
    ident_d = din("ident", [P, P], BF16)
    W_d = [din(f"W{li}", [P, P], BF16) for li in range(4)]
    bias_d = [din(f"bias{li}", [P, 1], F32) for li in range(4)]
    WihT_d = din("WihT", [P, 4 * P], BF16)
    WhhT_d = din("WhhT", [P, 4 * P], F32)
    bg_d = din("bg", [P, 4], F32)
    linW_d = din("linW", [P, cfg.C], F32)
    linb_d = din("linb", [P, cfg.C], F32)
    out_d = nc.dram_tensor("out", [NG, cfg.C], F32, kind="ExternalOutput")

    rg = [list(range(cfg.NCORES))]

    with tile.TileContext(nc) as tc:
        with tc.tile_pool(name="dram", bufs=1, space="DRAM") as dpool, \
             tc.tile_pool(name="const", bufs=1) as cpool, \
             tc.tile_pool(name="state", bufs=1) as spool, \
             tc.tile_pool(name="work", bufs=2) as wpool, \
             tc.tile_pool(name="psum", bufs=4, space="PSUM") as ppool:


            def cload(dram, shape, dt, tag):
                t = cpool.tile(shape, dt, tag=tag)
                nc.sync.dma_start(t[:], dram[:])
                return t
            iota_t = cload(iota_d, [P, P], BF16, "c_iota")
            ident_t = cload(ident_d, [P, P], BF16, "c_ident")
            W_t = [cload(W_d[i], [P, P], BF16, f"c_W{i}") for i in range(4)]
            bias_t = [cload(bias_d[i], [P, 1], F32, f"c_b{i}")
                      for i in range(4)]
            WihT_t = cload(WihT_d, [P, 4 * P], BF16, "c_wih")
            WhhT_t = cload(WhhT_d, [P, 4 * P], F32, "c_whh")
            bg_t = cload(bg_d, [P, 4], F32, "c_bg")
            linW_t = cload(linW_d, [P, cfg.C], F32, "c_linw")
            linb_t = cload(linb_d, [P, cfg.C], F32, "c_linb")

            a_t = spool.tile([P, SLAB], F32, tag="a")
            nc.sync.dma_start(a_t[:], a_d[:])

            t_even = spool.tile([P, SLAB], BF16, tag="t_even")
            t_big = spool.tile([P, 4 * SLAB], BF16, tag="t_big")
            slab_t = spool.tile([P, SLAB], BF16, tag="slab")

            for j in range(0, SLAB, 512):
                w = min(512, SLAB - j)
                xc = wpool.tile([P, 512], F32, tag="xchunk")
                nc.sync.dma_start(xc[:, :w], xT_d[:, j:j + w])
                nc.vector.tensor_tensor(
                    out=t_even[:, j:j + w], in0=xc[:, :w], in1=a_t[:, j:j + w],
                    op=mybir.AluOpType.mult)

            for li in range(4):
                cur = t_even if li % 2 == 0 else t_big
                nxt = t_big if li % 2 == 0 else t_even
                slab_dram = dpool.tile([SLAB, P], BF16, tag="slab_dram",
                                       bufs=2)
                table_dram = dpool.tile([cfg.NCORES * SLAB, P], BF16,
                                        addr_space="Shared", tag="table",
                                        bufs=2)

                # ---- u = t @ W -> slab [node, feat] ----
                for k in range(NBLK):
                    pu = ppool.tile([P, P], F32, tag="mm", space="PSUM")
                    nc.tensor.matmul(pu[:], lhsT=cur[:, k * P:(k + 1) * P],
                                     rhs=W_t[li][:], start=True, stop=True)
                    nc.vector.tensor_copy(out=slab_t[:, k * P:(k + 1) * P],
                                          in_=pu[:])
                nc.sync.dma_start(
                    slab_dram[:].rearrange("(b p) f -> p b f", p=P),
                    slab_t[:].rearrange("p (b f) -> p b f", f=P))
                if dbg_skip_coll:
                    nc.sync.dma_start(
                        table_dram[li % 2 * SLAB:(li % 2 + 1) * SLAB, :],
                        slab_dram[:])
                else:
                    nc.gpsimd.collective_compute(
                        "AllGather", mybir.AluOpType.bypass,
                        replica_groups=rg,
                        ins=[slab_dram[:]],
                        outs=[table_dram[:]],
                    )

                # ---- edge aggregation ----
                if li >= dbg_layers:
                    continue
                ao = 0
                bo = 0
                gq = 0
                if dbg_skip_edges:
                    continue
                for blks in groups:
                    nca = int(cA[blks].sum())
                    ncb = int(cB[blks].sum())
                    gx = {}
                    for half, ncnt, idxd, off16 in (
                            (0, nca, idxA_d, ao), (1, ncb, idxB_d, bo)):
                        if ncnt == 0:
                            continue
                        it = wpool.tile([P, ncnt * 8], I16, tag=f"idx{half}")
                        nc.sync.dma_start(
                            it[:], idxd[:, off16 * 8:(off16 + ncnt) * 8])
                        g = wpool.tile([P, ncnt, P], BF16, tag=f"gx{half}")
                        if dbg_skip_gather:
                            nc.vector.memset(g[:], 0.0)
                            gx[half] = g
                            continue
                        nc.gpsimd.dma_gather(
                            out_ap=g[:],
                            in_ap=table_dram[half * THALF:(half + 1) * THALF, :],
                            idxs_ap=it[:],
                            num_idxs=ncnt * P,
                            num_idxs_reg=ncnt * P,
                            elem_size=P,
                            single_packet=False,
                            queue_num=gq % 4,
                        )
                        gq += 1
                        gx[half] = g
                    sa = {}
                    for half, ncnt, segd, offc in (
                            (0, nca, segA_d, ao), (1, ncb, segB_d, bo)):
                        if ncnt == 0:
                            continue
                        st_ = wpool.tile([P, ncnt], BF16, tag=f"seg{half}")
                        nc.sync.dma_start(st_[:], segd[:, offc:offc + ncnt])
                        sa[half] = st_

                    ca_in_grp = 0
                    cb_in_grp = 0
                    for b in blks:
                        pb = ppool.tile([P, P], F32, tag="mm", space="PSUM")
                        na, nb_ = int(cA[b]), int(cB[b])
                        nc.tensor.matmul(pb[:],
                                         lhsT=slab_t[:, b * P:(b + 1) * P],
                                         rhs=ident_t[:], start=True,
                                         stop=(na + nb_ == 0))
                        done = 0
                        for half, cnt, base in ((0, na, ca_in_grp),
                                                (1, nb_, cb_in_grp)):
                            for ci in range(cnt):
                                col = base + ci
                                st = wpool.tile([P, P], BF16, tag="st",
                                                bufs=4)
                                nc.vector.tensor_tensor(
                                    out=st[:],
                                    in0=sa[half][:, col:col + 1]
                                        .to_broadcast((P, P)),
                                    in1=iota_t[:],
                                    op=mybir.AluOpType.is_equal)
                                done += 1
                                nc.tensor.matmul(
                                    pb[:], lhsT=gx[half][:, col, :], rhs=st[:],
                                    start=False, stop=(done == na + nb_))
                        ca_in_grp += na
                        cb_in_grp += nb_

                        tmp = wpool.tile([P, P], F32, tag="tmp")
                        nc.vector.tensor_tensor(
                            out=tmp[:], in0=pb[:],
                            in1=a_t[:, b * P:(b + 1) * P],
                            op=mybir.AluOpType.mult)
                        if li < 3:
                            z = wpool.tile([P, P], F32, tag="z")
                            nc.scalar.activation(
                                out=z[:], in_=tmp[:],
                                func=mybir.ActivationFunctionType.Relu,
                                bias=bias_t[li][:])
                            nc.vector.tensor_tensor(
                                out=nxt[:, b * P:(b + 1) * P], in0=z[:],
                                in1=a_t[:, b * P:(b + 1) * P],
                                op=mybir.AluOpType.mult)
                        else:
                            nc.scalar.activation(
                                out=nxt[:, b * P:(b + 1) * P], in_=tmp[:],
                                func=mybir.ActivationFunctionType.Relu,
                                bias=bias_t[li][:])
                    ao += nca
                    bo += ncb

            z4 = t_even
            gatesx = t_big  # reuse [P, 4*SLAB] bf16

            for q in range(4) if not dbg_skip_lstm else []:
                for j in range(0, SLAB, 512):
                    w = min(512, SLAB - j)
                    pgx = ppool.tile([P, 512], F32, tag="mm", space="PSUM")
                    nc.tensor.matmul(
                        pgx[:, :w], lhsT=WihT_t[:, q * P:(q + 1) * P],
                        rhs=z4[:, j:j + w], start=True, stop=True)
                    nc.vector.tensor_copy(
                        out=gatesx[:, q * SLAB + j:q * SLAB + j + w],
                        in_=pgx[:, :w])

            if dbg_skip_lstm:
                dummy = wpool.tile([P, cfg.C], F32, tag="outs")
                nc.vector.memset(dummy[:], 0.0)
                nc.sync.dma_start(out_d[:], dummy[:NG, :])
            c_t = spool.tile([P, NG], F32, tag="c")
            h_t = spool.tile([P, NG], F32, tag="h")
            nc.vector.memset(c_t[:], 0.0)
            nc.vector.memset(h_t[:], 0.0)

            for t in range(cfg.L) if not dbg_skip_lstm else []:
                pg = ppool.tile([P, 4 * NG], F32, tag="lstm", space="PSUM",
                                bufs=2)
                for q in range(4):
                    nc.tensor.matmul(
                        pg[:, q * NG:(q + 1) * NG],
                        lhsT=WhhT_t[:, q * P:(q + 1) * P],
                        rhs=h_t[:], start=True, stop=True)
                gsum = wpool.tile([P, 4 * NG], F32, tag="gsum")
                for q in range(4):
                    nc.vector.tensor_tensor(
                        out=gsum[:, q * NG:(q + 1) * NG],
                        in0=pg[:, q * NG:(q + 1) * NG],
                        in1=gatesx[:, q * SLAB + t:q * SLAB + cfg.S_PAD:cfg.L],
                        op=mybir.AluOpType.add)
                acts = []
                for q, fn in enumerate((
                        mybir.ActivationFunctionType.Sigmoid,
                        mybir.ActivationFunctionType.Sigmoid,
                        mybir.ActivationFunctionType.Tanh,
                        mybir.ActivationFunctionType.Sigmoid)):
                    av = wpool.tile([P, NG], F32, tag=f"act{q}")
                    nc.scalar.activation(
                        out=av[:], in_=gsum[:, q * NG:(q + 1) * NG],
                        func=fn, bias=bg_t[:, q:q + 1])
                    acts.append(av)
                i_t, f_t, g_t, o_t = acts
                fc = wpool.tile([P, NG], F32, tag="fc")
                nc.vector.tensor_tensor(out=fc[:], in0=f_t[:], in1=c_t[:],
                                        op=mybir.AluOpType.mult)
                ig = wpool.tile([P, NG], F32, tag="ig")
                nc.vector.tensor_tensor(out=ig[:], in0=i_t[:], in1=g_t[:],
                                        op=mybir.AluOpType.mult)
                nc.vector.tensor_tensor(out=c_t[:], in0=fc[:], in1=ig[:],
                                        op=mybir.AluOpType.add)
                tc_ = wpool.tile([P, NG], F32, tag="tc")
                nc.scalar.activation(out=tc_[:], in_=c_t[:],
                                     func=mybir.ActivationFunctionType.Tanh)
                nc.vector.tensor_tensor(out=h_t[:], in0=o_t[:], in1=tc_[:],
                                        op=mybir.AluOpType.mult)

            if not dbg_skip_lstm:
                po = ppool.tile([P, cfg.C], F32, tag="lstm", space="PSUM",
                                bufs=2)
                nc.tensor.matmul(po[:NG, :], lhsT=h_t[:, :NG], rhs=linW_t[:],
                                 start=True, stop=True)
                os_ = wpool.tile([P, cfg.C], F32, tag="outs")
                nc.vector.tensor_tensor(out=os_[:NG, :], in0=po[:NG, :],
                                        in1=linb_t[:NG, :],
                                        op=mybir.AluOpType.add)
                nc.sync.dma_start(out_d[:], os_[:NG, :])

    nc.compile()
    return nc


def assemble(cfg, results):
    out = np.zeros((cfg.G, cfg.C), np.float32)
    for c in range(cfg.NCORES):
        g0 = int(cfg.offs[c]) // cfg.L
        ng = cfg.sizes[c] // cfg.L
        out[g0:g0 + ng] = results[c]["out"][:ng]
    return out


_BUILD_CACHE = {}


def kernel(x, edge_index, batch, W1, b1, W2, b2, W3, b3, W4, b4,
           W_ih, W_hh, b_ih, b_hh, lin_W, lin_b):
    global LAST_RESULTS
    cfg = Config()
    x = np.asarray(x, np.float32)
    edge_index = np.asarray(edge_index, np.int64)
    Ws = [np.asarray(w, np.float32) for w in (W1, W2, W3, W4)]
    bs = [np.asarray(b, np.float32) for b in (b1, b2, b3, b4)]

    in_maps, sched = preprocess(
        cfg, x, edge_index, Ws, bs,
        np.asarray(W_ih, np.float32), np.asarray(W_hh, np.float32),
        np.asarray(b_ih, np.float32), np.asarray(b_hh, np.float32),
        np.asarray(lin_W, np.float32), np.asarray(lin_b, np.float32))

    key = (sched["TA"], sched["TB"], tuple(sched["cA"]), tuple(sched["cB"]))
    if key not in _BUILD_CACHE:
        _BUILD_CACHE[key] = build_program(cfg, sched)
    nc = _BUILD_CACHE[key]

    res = run_bass_kernel_spmd(nc, in_maps, core_ids=list(range(cfg.NCORES)),
                               trace=TRACE)
    LAST_RESULTS = res
    return assemble(cfg, res.results)


# revision 9
# speedup vs baseline: 1.4924x; 1.0173x over previous
"""Distributed GCN(4-layer) + LSTM readout kernel for 8 TRN2 NeuronCores.

Self-contained: hardcodes the problem shapes (N=50000, E=800000, D=H=128,
G=500 graphs x L=100 nodes, C=10) and the 8-way sharding.

Strategy
--------
- Nodes are sharded contiguously across 8 cores at graph boundaries
  (sizes 6300 x4 + 6200 x4), so the per-graph LSTM readout is purely local.
- Per GCN layer, each core computes u = t @ W for its shard in [node, feat]
  layout (t carries the deg^-1/2 row prescale: norm separability
  a[src]*a[dst] -> prescale rows, post-scale dst blocks), writes the slab to
  DRAM and AllGathers the 8 slabs into a replicated bf16 table.
- Edge aggregation: edges partitioned by dst shard, grouped by 128-dst
  blocks. The scatter-add becomes PSUM matmul accumulation:
  psum[f,d] += sum_e GX[e,f] * S[e,d], with GX = dma_gather of table rows
  (by src) and S a 0/1 staircase built on the vector engine via
  is_equal(seg, iota). The self-loop enters as one identity matmul of the
  local slab block. Post per block: h = relu(a*psum + b), t_next = a*h.
- dma_gather indices are int16, so the table is addressed in two halves
  (cores 0-3 / 4-7) and per-block edge lists are split accordingly.
- LSTM: x-projections for all timesteps are 4 big matmuls up front; the
  recurrence runs 4 [128x128]@[128,63] matmuls + gate math per step.
"""
import dataclasses
import os
import numpy as np
import ml_dtypes

import concourse.bass as bass
import concourse.mybir as mybir
import concourse.tile as tile
from concourse import bacc
from concourse.bass_utils import run_bass_kernel_spmd

F32 = mybir.dt.float32
BF16 = mybir.dt.bfloat16
I16 = mybir.dt.int16
P = 128

TRACE = False          # set True (e.g. from test.py) to profile
LAST_RESULTS = None    # BassKernelResults of the last run (for profiling)


@dataclasses.dataclass
class Config:
    N: int = 50000
    E: int = 800000
    D: int = 128
    H: int = 128
    L: int = 100
    C: int = 10
    NCORES: int = 8
    GROUP_BLOCKS: int = 3  # dst blocks per gather super-group

    def __post_init__(self):
        assert self.D == 128 and self.H == 128
        base = (self.N // self.NCORES) // self.L * self.L
        hi = base + self.L
        n_hi = (self.N - base * self.NCORES) // self.L
        self.sizes = [hi] * n_hi + [base] * (self.NCORES - n_hi)
        assert sum(self.sizes) == self.N
        self.offs = np.concatenate([[0], np.cumsum(self.sizes)]).astype(np.int64)
        self.S_PAD = hi
        self.NBLK = -(-self.S_PAD // P)
        self.SLAB = self.NBLK * P
        self.THALF = (self.NCORES // 2) * self.SLAB
        assert self.THALF <= 32768, "int16 gather index overflow"
        self.NG = self.S_PAD // self.L
        self.G = self.N // self.L


PAD_SEG = 255.0


def preprocess(cfg, x, edge_index, Ws, bs, W_ih, W_hh, b_ih, b_hh,
               lin_W, lin_b):
    N = cfg.N
    src = np.asarray(edge_index[0], np.int64)
    dst = np.asarray(edge_index[1], np.int64)
    deg = (np.bincount(dst, minlength=N) + 1.0).astype(np.float32)
    a = (1.0 / np.sqrt(deg)).astype(np.float32)

    shard_of = np.searchsorted(cfg.offs[1:], np.arange(N), side="right")
    trow = shard_of * cfg.SLAB + (np.arange(N) - cfg.offs[shard_of])

    e_core = shard_of[dst]
    e_half = (trow[src] >= cfg.THALF).astype(np.int64)
    e_tix = (trow[src] - e_half * cfg.THALF).astype(np.int64)
    e_blk = ((dst - cfg.offs[e_core]) // P).astype(np.int64)
    e_seg = ((dst - cfg.offs[e_core]) % P).astype(np.int64)

    order = np.lexsort((dst, e_blk, e_half, e_core))
    src_s = src[order]
    core_s, half_s, tix_s, blk_s, seg_s = (
        arr[order] for arr in (e_core, e_half, e_tix, e_blk, e_seg))

    counts = np.zeros((cfg.NCORES, 2, cfg.NBLK), np.int64)
    np.add.at(counts, (core_s, half_s, blk_s), 1)
    chunks = -(-counts.max(axis=0) // P)
    cA, cB = chunks[0], chunks[1]

    groups = []
    for g0 in range(0, cfg.NBLK, cfg.GROUP_BLOCKS):
        groups.append(list(range(g0, min(g0 + cfg.GROUP_BLOCKS, cfg.NBLK))))

    TA = int(cA.sum()) * P
    TB = int(cB.sum()) * P
    run_off = np.zeros((cfg.NCORES, 2, cfg.NBLK), np.int64)
    run_off.reshape(-1)[1:] = np.cumsum(counts.reshape(-1))[:-1]

    in_maps = []
    for c in range(cfg.NCORES):
        idx_flat = {0: np.zeros(TA, np.int64), 1: np.zeros(TB, np.int64)}
        seg_flat = {0: np.full(TA, PAD_SEG, np.float32),
                    1: np.full(TB, PAD_SEG, np.float32)}
        for h_i, tot_c in enumerate((cA, cB)):
            pos = 0
            for b in range(cfg.NBLK):
                n = int(counts[c, h_i, b])
                o = int(run_off[c, h_i, b])
                idx_flat[h_i][pos:pos + n] = tix_s[o:o + n]
                seg_flat[h_i][pos:pos + n] = seg_s[o:o + n]
                pos += int(tot_c[b]) * P
        idxA = np.zeros((P, max(TA // 16, 1)), np.int16)
        idxB = np.zeros((P, max(TB // 16, 1)), np.int16)
        if TA:
            idxA[:] = np.tile(idx_flat[0].reshape(-1, 16).T.astype(np.int16),
                              (8, 1))
        if TB:
            idxB[:] = np.tile(idx_flat[1].reshape(-1, 16).T.astype(np.int16),
                              (8, 1))
        segA = np.zeros((P, max(TA // P, 1)), ml_dtypes.bfloat16)
        segB = np.zeros((P, max(TB // P, 1)), ml_dtypes.bfloat16)
        if TA:
            segA[:] = seg_flat[0].reshape(-1, P).T.astype(ml_dtypes.bfloat16)
        if TB:
            segB[:] = seg_flat[1].reshape(-1, P).T.astype(ml_dtypes.bfloat16)

        o, s = int(cfg.offs[c]), int(cfg.sizes[c])
        xT = np.zeros((P, cfg.SLAB), np.float32)
        xT[:, :s] = np.asarray(x[o:o + s], np.float32).T
        a_b = np.ones((P, cfg.SLAB), ml_dtypes.bfloat16)
        a_b[:, :s] = np.tile(a[o:o + s], (P, 1)).astype(ml_dtypes.bfloat16)

        m = {
            "xT": xT, "a_b": a_b,
            "idxA": idxA, "idxB": idxB, "segA": segA, "segB": segB,
            "iota": np.tile(np.arange(P, dtype=np.float32), (P, 1)).astype(
                ml_dtypes.bfloat16),
            "ident": np.eye(P, dtype=ml_dtypes.bfloat16),
            "WihT": np.ascontiguousarray(
                np.asarray(W_ih, np.float32).T).astype(ml_dtypes.bfloat16),
            "WhhT": np.ascontiguousarray(np.asarray(W_hh, np.float32).T),
            "bg": np.ascontiguousarray(
                (np.asarray(b_ih, np.float32)
                 + np.asarray(b_hh, np.float32)).reshape(4, P).T),
            "linW": np.asarray(lin_W, np.float32),
            "linb": np.tile(np.asarray(lin_b, np.float32), (P, 1)),
        }
        for li in range(4):
            m[f"W{li}"] = np.asarray(Ws[li], np.float32).astype(
                ml_dtypes.bfloat16)
            m[f"bias{li}"] = np.asarray(bs[li], np.float32).reshape(P, 1)
        in_maps.append(m)

    sched = dict(cA=cA.astype(np.int64), cB=cB.astype(np.int64),
                 groups=groups, TA=TA, TB=TB)
    return in_maps, sched


def build_program(cfg, sched, trn_type="TRN2", debug=False):
    dbg_layers = int(os.environ.get("GNN_LAYERS", "4"))
    dbg_skip_edges = os.environ.get("GNN_SKIP_EDGES", "0") == "1"
    dbg_skip_gather = os.environ.get("GNN_SKIP_GATHER", "0") == "1"
    dbg_skip_lstm = os.environ.get("GNN_SKIP_LSTM", "0") == "1"
    dbg_skip_coll = os.environ.get("GNN_SKIP_COLL", "0") == "1"
    nc = bacc.Bacc(trn_type, target_bir_lowering=False, debug=debug,
                   num_devices=cfg.NCORES, num_swdge_queues=4)
    cA, cB, groups = sched["cA"], sched["cB"], sched["groups"]
    TA, TB = sched["TA"], sched["TB"]
    SLAB, NBLK, THALF, NG = cfg.SLAB, cfg.NBLK, cfg.THALF, cfg.NG

    def din(name, shape, dt):
        return nc.dram_tensor(name, shape, dt, kind="ExternalInput")

    xT_d = din("xT", [P, SLAB], F32)
    a_d = din("a_b", [P, SLAB], BF16)
    idxA_d = din("idxA", [P, max(TA // 16, 1)], I16)
    idxB_d = din("idxB", [P, max(TB // 16, 1)], I16)
    segA_d = din("segA", [P, max(TA // P, 1)], BF16)
    segB_d = din("segB", [P, max(TB // P, 1)], BF16)
    iota_d = din("iota", [P, P], BF16)
    ident_d = din("ident", [P, P], BF16)
    W_d = [din(f"W{li}", [P, P], BF16) for li in range(4)]
    bias_d = [din(f"bias{li}", [P, 1], F32) for li in range(4)]
    WihT_d = din("WihT", [P, 4 * P], BF16)
    WhhT_d = din("WhhT", [P, 4 * P], F32)
    bg_d = din("bg", [P, 4], F32)
    linW_d = din("linW", [P, cfg.C], F32)
    linb_d = din("linb", [P, cfg.C], F32)
    out_d = nc.dram_tensor("out", [NG, cfg.C], F32, kind="ExternalOutput")

    rg = [list(range(cfg.NCORES))]

    with tile.TileContext(nc) as tc:
        with tc.tile_pool(name="dram", bufs=1, space="DRAM") as dpool, \
             tc.tile_pool(name="const", bufs=1) as cpool, \
             tc.tile_pool(name="state", bufs=1) as spool, \
             tc.tile_pool(name="work", bufs=2) as wpool, \
             tc.tile_pool(name="psum", bufs=4, space="PSUM") as ppool:


            def cload(dram, shape, dt, tag):
                t = cpool.tile(shape, dt, tag=tag)
                nc.sync.dma_start(t[:], dram[:])
                return t
            iota_t = cload(iota_d, [P, P], BF16, "c_iota")
            ident_t = cload(ident_d, [P, P], BF16, "c_ident")
            W_t = [cload(W_d[i], [P, P], BF16, f"c_W{i}") for i in range(4)]
            bias_t = [cload(bias_d[i], [P, 1], F32, f"c_b{i}")
                      for i in range(4)]
            WihT_t = cload(WihT_d, [P, 4 * P], BF16, "c_wih")
            WhhT_t = cload(WhhT_d, [P, 4 * P], F32, "c_whh")
            bg_t = cload(bg_d, [P, 4], F32, "c_bg")
            linW_t = cload(linW_d, [P, cfg.C], F32, "c_linw")
            linb_t = cload(linb_d, [P, cfg.C], F32, "c_linb")

            a_t = spool.tile([P, SLAB], BF16, tag="a")
            nc.sync.dma_start(a_t[:], a_d[:])

            t_even = spool.tile([P, SLAB], BF16, tag="t_even")
            t_big = spool.tile([P, 4 * SLAB], BF16, tag="t_big")
            slab_t = spool.tile([P, SLAB], BF16, tag="slab")

            for j in range(0, SLAB, 512):
                w = min(512, SLAB - j)
                xc = wpool.tile([P, 512], F32, tag="xchunk")
                nc.sync.dma_start(xc[:, :w], xT_d[:, j:j + w])
                nc.vector.tensor_tensor(
                    out=t_even[:, j:j + w], in0=xc[:, :w], in1=a_t[:, j:j + w],
                    op=mybir.AluOpType.mult)

            for li in range(4):
                cur = t_even if li % 2 == 0 else t_big
                nxt = t_big if li % 2 == 0 else t_even
                slab_dram = dpool.tile([SLAB, P], BF16, tag="slab_dram",
                                       bufs=2)
                table_dram = dpool.tile([cfg.NCORES * SLAB, P], BF16,
                                        addr_space="Shared", tag="table",
                                        bufs=2)

                # ---- u = t @ W -> slab [node, feat] ----
                for k in range(NBLK):
                    pu = ppool.tile([P, P], F32, tag="mm", space="PSUM")
                    nc.tensor.matmul(pu[:], lhsT=cur[:, k * P:(k + 1) * P],
                                     rhs=W_t[li][:], start=True, stop=True)
                    nc.vector.tensor_copy(out=slab_t[:, k * P:(k + 1) * P],
                                          in_=pu[:])
                nc.sync.dma_start(
                    slab_dram[:].rearrange("(b p) f -> p b f", p=P),
                    slab_t[:].rearrange("p (b f) -> p b f", f=P))
                if dbg_skip_coll:
                    nc.sync.dma_start(
                        table_dram[li % 2 * SLAB:(li % 2 + 1) * SLAB, :],
                        slab_dram[:])
                else:
                    nc.gpsimd.collective_compute(
                        "AllGather", mybir.AluOpType.bypass,
                        replica_groups=rg,
                        ins=[slab_dram[:]],
                        outs=[table_dram[:]],
                    )

                # ---- edge aggregation ----
                if li >= dbg_layers:
                    continue
                ao = 0
                bo = 0
                gq = 0
                if dbg_skip_edges:
                    continue
                for blks in groups:
                    nca = int(cA[blks].sum())
                    ncb = int(cB[blks].sum())
                    gx = {}
                    for half, ncnt, idxd, off16 in (
                            (0, nca, idxA_d, ao), (1, ncb, idxB_d, bo)):
                        if ncnt == 0:
                            continue
                        it = wpool.tile([P, ncnt * 8], I16, tag=f"idx{half}")
                        nc.sync.dma_start(
                            it[:], idxd[:, off16 * 8:(off16 + ncnt) * 8])
                        g = wpool.tile([P, ncnt, P], BF16, tag=f"gx{half}")
                        if dbg_skip_gather:
                            nc.vector.memset(g[:], 0.0)
                            gx[half] = g
                            continue
                        nc.gpsimd.dma_gather(
                            out_ap=g[:],
                            in_ap=table_dram[half * THALF:(half + 1) * THALF, :],
                            idxs_ap=it[:],
                            num_idxs=ncnt * P,
                            num_idxs_reg=ncnt * P,
                            elem_size=P,
                            single_packet=False,
                            queue_num=gq % 4,
                        )
                        gq += 1
                        gx[half] = g
                    stg = {}
                    for half, ncnt, segd, offc in (
                            (0, nca, segA_d, ao), (1, ncb, segB_d, bo)):
                        if ncnt == 0:
                            continue
                        st_ = wpool.tile([P, ncnt], BF16, tag=f"seg{half}")
                        nc.sync.dma_start(st_[:], segd[:, offc:offc + ncnt])
                        sg = wpool.tile([P, ncnt, P], BF16, tag=f"stg{half}")
                        nc.vector.tensor_tensor(
                            out=sg[:],
                            in0=st_[:].rearrange("p (c o) -> p c o", o=1)
                                .to_broadcast((P, ncnt, P)),
                            in1=iota_t[:].rearrange("p (o f) -> p o f", o=1)
                                .to_broadcast((P, ncnt, P)),
                            op=mybir.AluOpType.is_equal)
                        stg[half] = sg

                    ca_in_grp = 0
                    cb_in_grp = 0
                    for b in blks:
                        pb = ppool.tile([P, P], F32, tag="mm", space="PSUM")
                        na, nb_ = int(cA[b]), int(cB[b])
                        nc.tensor.matmul(pb[:],
                                         lhsT=slab_t[:, b * P:(b + 1) * P],
                                         rhs=ident_t[:], start=True,
                                         stop=(na + nb_ == 0))
                        done = 0
                        for half, cnt, base in ((0, na, ca_in_grp),
                                                (1, nb_, cb_in_grp)):
                            for ci in range(cnt):
                                col = base + ci
                                done += 1
                                nc.tensor.matmul(
                                    pb[:], lhsT=gx[half][:, col, :],
                                    rhs=stg[half][:, col, :],
                                    start=False, stop=(done == na + nb_))
                        ca_in_grp += na
                        cb_in_grp += nb_

                        tmp = wpool.tile([P, P], F32, tag="tmp")
                        nc.vector.tensor_tensor(
                            out=tmp[:], in0=pb[:],
                            in1=a_t[:, b * P:(b + 1) * P],
                            op=mybir.AluOpType.mult)
                        if li < 3:
                            z = wpool.tile([P, P], F32, tag="z")
                            nc.scalar.activation(
                                out=z[:], in_=tmp[:],
                                func=mybir.ActivationFunctionType.Relu,
                                bias=bias_t[li][:])
                            nc.vector.tensor_tensor(
                                out=nxt[:, b * P:(b + 1) * P], in0=z[:],
                                in1=a_t[:, b * P:(b + 1) * P],
                                op=mybir.AluOpType.mult)
                        else:
                            nc.scalar.activation(
                                out=nxt[:, b * P:(b + 1) * P], in_=tmp[:],
                                func=mybir.ActivationFunctionType.Relu,
                                bias=bias_t[li][:])
                    ao += nca
                    bo += ncb

            z4 = t_even
            gatesx = t_big  # reuse [P, 4*SLAB] bf16

            for q in range(4) if not dbg_skip_lstm else []:
                for j in range(0, SLAB, 512):
                    w = min(512, SLAB - j)
                    pgx = ppool.tile([P, 512], F32, tag="mm", space="PSUM")
                    nc.tensor.matmul(
                        pgx[:, :w], lhsT=WihT_t[:, q * P:(q + 1) * P],
                        rhs=z4[:, j:j + w], start=True, stop=True)
                    nc.vector.tensor_copy(
                        out=gatesx[:, q * SLAB + j:q * SLAB + j + w],
                        in_=pgx[:, :w])

            if dbg_skip_lstm:
                dummy = wpool.tile([P, cfg.C], F32, tag="outs")
                nc.vector.memset(dummy[:], 0.0)
                nc.sync.dma_start(out_d[:], dummy[:NG, :])
            c_t = spool.tile([P, NG], F32, tag="c")
            h_t = spool.tile([P, NG], F32, tag="h")
            nc.vector.memset(c_t[:], 0.0)
            nc.vector.memset(h_t[:], 0.0)

            for t in range(cfg.L) if not dbg_skip_lstm else []:
                pg = ppool.tile([P, 4 * NG], F32, tag="lstm", space="PSUM",
                                bufs=2)
                for q in range(4):
                    nc.tensor.matmul(
                        pg[:, q * NG:(q + 1) * NG],
                        lhsT=WhhT_t[:, q * P:(q + 1) * P],
                        rhs=h_t[:], start=True, stop=True)
                gsum = wpool.tile([P, 4 * NG], F32, tag="gsum")
                for q in range(4):
                    nc.vector.tensor_tensor(
                        out=gsum[:, q * NG:(q + 1) * NG],
                        in0=pg[:, q * NG:(q + 1) * NG],
                        in1=gatesx[:, q * SLAB + t:q * SLAB + cfg.S_PAD:cfg.L],
                        op=mybir.AluOpType.add)
                acts = []
                for q, fn in enumerate((
                        mybir.ActivationFunctionType.Sigmoid,
                        mybir.ActivationFunctionType.Sigmoid,
                        mybir.ActivationFunctionType.Tanh,
                        mybir.ActivationFunctionType.Sigmoid)):
                    av = wpool.tile([P, NG], F32, tag=f"act{q}")
                    nc.scalar.activation(
                        out=av[:], in_=gsum[:, q * NG:(q + 1) * NG],
                        func=fn, bias=bg_t[:, q:q + 1])
                    acts.append(av)
                i_t, f_t, g_t, o_t = acts
                fc = wpool.tile([P, NG], F32, tag="fc")
                nc.vector.tensor_tensor(out=fc[:], in0=f_t[:], in1=c_t[:],
                                        op=mybir.AluOpType.mult)
                ig = wpool.tile([P, NG], F32, tag="ig")
                nc.vector.tensor_tensor(out=ig[:], in0=i_t[:], in1=g_t[:],
                                        op=mybir.AluOpType.mult)
                nc.vector.tensor_tensor(out=c_t[:], in0=fc[:], in1=ig[:],
                                        op=mybir.AluOpType.add)
                tc_ = wpool.tile([P, NG], F32, tag="tc")
                nc.scalar.activation(out=tc_[:], in_=c_t[:],
                                     func=mybir.ActivationFunctionType.Tanh)
                nc.vector.tensor_tensor(out=h_t[:], in0=o_t[:], in1=tc_[:],
                                        op=mybir.AluOpType.mult)

            if not dbg_skip_lstm:
                po = ppool.tile([P, cfg.C], F32, tag="lstm", space="PSUM",
                                bufs=2)
                nc.tensor.matmul(po[:NG, :], lhsT=h_t[:, :NG], rhs=linW_t[:],
                                 start=True, stop=True)
                os_ = wpool.tile([P, cfg.C], F32, tag="outs")
                nc.vector.tensor_tensor(out=os_[:NG, :], in0=po[:NG, :],
                                        in1=linb_t[:NG, :],
                                        op=mybir.AluOpType.add)
                nc.sync.dma_start(out_d[:], os_[:NG, :])

    nc.compile()
    return nc


def assemble(cfg, results):
    out = np.zeros((cfg.G, cfg.C), np.float32)
    for c in range(cfg.NCORES):
        g0 = int(cfg.offs[c]) // cfg.L
        ng = cfg.sizes[c] // cfg.L
        out[g0:g0 + ng] = results[c]["out"][:ng]
    return out


_BUILD_CACHE = {}


def kernel(x, edge_index, batch, W1, b1, W2, b2, W3, b3, W4, b4,
           W_ih, W_hh, b_ih, b_hh, lin_W, lin_b):
    global LAST_RESULTS
    cfg = Config()
    x = np.asarray(x, np.float32)
    edge_index = np.asarray(edge_index, np.int64)
    Ws = [np.asarray(w, np.float32) for w in (W1, W2, W3, W4)]
    bs = [np.asarray(b, np.float32) for b in (b1, b2, b3, b4)]

    in_maps, sched = preprocess(
        cfg, x, edge_index, Ws, bs,
        np.asarray(W_ih, np.float32), np.asarray(W_hh, np.float32),
        np.asarray(b_ih, np.float32), np.asarray(b_hh, np.float32),
        np.asarray(lin_W, np.float32), np.asarray(lin_b, np.float32))

    key = (sched["TA"], sched["TB"], tuple(sched["cA"]), tuple(sched["cB"]))
    if key not in _BUILD_CACHE:
        _BUILD_CACHE[key] = build_program(cfg, sched)
    nc = _BUILD_CACHE[key]

    res = run_bass_kernel_spmd(nc, in_maps, core_ids=list(range(cfg.NCORES)),
                               trace=TRACE)
    LAST_RESULTS = res
    return assemble(cfg, res.results)
